# revision 17
# baseline (speedup 1.0000x reference)
"""Trainium2 Bass kernel for nn_Attention_62715112456978.

The reference attention is algebraically rank-1: keys/queries/values are
outer products x (x) w, so

    dot[b,q,k]   = c_b * x[b,q] * x[b,k],   c_b = sum_e wq*wk / sqrt(e)
    softmax-out  = m[b,q] * wv[b,:],        m[b,q] = sum_k A[b,q,k]*x[b,k]
    final        = elu(m[b,q] * r_b + v[b,q]),  r_b = sum_e wv*wo

with wq/wk/wv/wo = |state @ W.T + b| (only the products c, r are needed,
and |a|*|b| = |a*b|, so the abs never has to be materialized).

Per 128-batch chunk (big ops have free size 64q*64k = 4096):
  PE:   hypernet matmuls in split-bf16 (hi+res decomposition of both
        state.T and the weights ~ fp32 accuracy at bf16 speed; biases are
        prefilled by a K=2 bf16 matmul of [bias_hi; bias_res])
  DVE:  Lraw = xq*xk in fp16 (duplicated-pair operand layout keeps every
        last AP dim packed -> 2x mode), EX = E*x at 2x, the two fold-1
        adds, and a joint fp32 reduce of both fold tails
  Act:  E = exp(c*Lraw - 40) with per-batch scale c as the activation's
        per-partition scale; relu/exp pieces of the elu tail
  Pool: hypernet product pq, the fold-2/fold-3 adds of both paths
The 1/sqrt(e) softmax scale is folded into wk/wq on the host. The
recip/m/z/elu tail runs batched per 2-chunk group with the fast
reciprocal; inputs arrive in one DMA per tensor.

Sharding: pure data parallel over batch; 8 cores x 512 batches each.
"""

import numpy as np

import concourse.bacc as bacc
import concourse.bass as bass
import concourse.tile as tile
from concourse import mybir
from concourse.bass_utils import run_bass_kernel_spmd

F32 = mybir.dt.float32
F16 = mybir.dt.float16
BF16 = mybir.dt.bfloat16

N_CORES = 8
B_FULL = 4096
BC = B_FULL // N_CORES  # 512 batches per core
CH = 128                # batches per chunk (partition dim)
NCH = BC // CH          # 4 chunks per core
T = 64                  # sequence length
D = 128                 # d_state
NW = 5 * 128 - 64       # 576 = wk|wv|wq|wo (128 each) + V (64) output columns
C_SHIFT = 40.0          # global exp shift; cancels in softmax, avoids overflow

_compiled = {}
_last_in_maps = None


def _build():
    nc = bacc.Bacc("TRN2", target_bir_lowering=False, debug=False,
                   num_devices=N_CORES)
    xalld = nc.dram_tensor("xall", [BC, 3 * T], F16, kind="ExternalInput")
    stTd = nc.dram_tensor("stT", [2, D, BC], BF16, kind="ExternalInput")
    wd = nc.dram_tensor("wcatT", [2, D, NW], BF16, kind="ExternalInput")
    bd = nc.dram_tensor("biascat", [2, NW], BF16, kind="ExternalInput")
    onesd = nc.dram_tensor("ones2", [2, CH], BF16, kind="ExternalInput")
    od = nc.dram_tensor("out", [BC, T], F32, kind="ExternalOutput")

    with tile.TileContext(nc) as tc:
        with (
            tc.tile_pool(name="const", bufs=1) as cpool,
            tc.tile_pool(name="big", bufs=4) as bigp,
            tc.tile_pool(name="small", bufs=2) as smp,
            tc.tile_pool(name="psum_h", bufs=2, space="PSUM") as psh,
        ):
            # x first on the fast SP queue (feeds the L-multiplies), the
            # big weight tensor + remaining state via the otherwise-idle
            # Pool queue's software DGE, in parallel
            xall = cpool.tile([CH, NCH, 3 * T], F16)
            nc.sync.dma_start(
                xall[:], xalld[:].rearrange("(c p) t -> p c t", c=NCH))
            stTr = stTd[:].rearrange("s d b -> d s b")
            stT = cpool.tile([D, 2, BC], BF16)
            nc.sync.dma_start(stT[:, :, 0:CH], stTr[:, :, 0:CH])
            biascat = cpool.tile([2, NW], BF16)
            nc.sync.dma_start(biascat[:], bd[:])
            ones2 = cpool.tile([2, CH], BF16)
            nc.sync.dma_start(ones2[:], onesd[:])
            wcat = cpool.tile([D, 2, NW], BF16)
            nc.gpsimd.dma_start(wcat[:], wd[:].rearrange("s d w -> d s w"))
            nc.gpsimd.dma_start(stT[:, :, CH:BC], stTr[:, :, CH:BC])
            shift = cpool.tile([CH, 1], F32)
            nc.vector.memset(shift[:], -C_SHIFT)

            # cross-chunk accumulators for the grouped tail
            cr_all = cpool.tile([CH, NCH, 2], F32)
            v_all = cpool.tile([CH, NCH, T], F32)
            dnm_all = cpool.tile([CH, 2, NCH, T], F32)  # [denom | numer]

            def tail_group(g):
                """z = (nm/dn)*r + v; out = elu(z), for chunks 2g, 2g+1."""
                cs = slice(2 * g, 2 * g + 2)
                dn = dnm_all[:, 0, cs, :]
                nm = dnm_all[:, 1, cs, :]
                dinv = smp.tile([CH, 2, T], F32, tag="dinv")
                nc.vector.reciprocal_approx_fast(dinv[:], dn)
                m_sb = smp.tile([CH, 2, T], F32, tag="m")
                nc.vector.tensor_tensor(m_sb[:], nm, dinv[:],
                                        op=mybir.AluOpType.mult)
                mr = smp.tile([CH, 2, T], F32, tag="mr")
                r_b = cr_all[:, cs, 1:2].broadcast_to([CH, 2, T])
                nc.vector.tensor_tensor(mr[:], m_sb[:], r_b,
                                        op=mybir.AluOpType.mult)
                z = smp.tile([CH, 2, T], F32, tag="z")
                nc.vector.tensor_tensor(z[:], mr[:], v_all[:, cs, :],
                                        op=mybir.AluOpType.add)
                zp = smp.tile([CH, 2, T], F32, tag="zp")
                nc.scalar.activation(zp[:], z[:],
                                     mybir.ActivationFunctionType.Relu)
                yn = smp.tile([CH, 2, T], F32, tag="yn")
                nc.scalar.activation(yn[:], z[:],
                                     mybir.ActivationFunctionType.Relu,
                                     scale=-1.0)
                ez = smp.tile([CH, 2, T], F32, tag="ez")
                nc.scalar.activation(ez[:], yn[:],
                                     mybir.ActivationFunctionType.Exp,
                                     scale=-1.0)
                o_sb = smp.tile([CH, 2, T], F32, tag="o")
                nc.vector.scalar_tensor_tensor(o_sb[:], zp[:], -1.0, ez[:],
                                               op0=mybir.AluOpType.add,
                                               op1=mybir.AluOpType.add)
                dst = od[2 * g * CH:(2 * g + 2) * CH, :]
                nc.sync.dma_start(
                    dst.rearrange("(c p) t -> p c t", c=2), o_sb[:])

            L_tiles = []
            W_tiles = []
            for ci in range(NCH):
                bs = ci * CH

                # hypernet: hy[b, j] = state @ wcat + bias, in split bf16:
                # bias (K=2: hi+res rows), then sh*wh + sh*wr + sr*wh.
                hy0 = psh.tile([CH, 512], F32, tag="hy0")
                nc.tensor.matmul(hy0[:], ones2[:], biascat[:, 0:512],
                                 start=True, stop=False)
                nc.tensor.matmul(hy0[:], stT[:, 0, bs:bs + CH],
                                 wcat[:, 0, 0:512], start=False, stop=False)
                nc.tensor.matmul(hy0[:], stT[:, 0, bs:bs + CH],
                                 wcat[:, 1, 0:512], start=False, stop=False)
                nc.tensor.matmul(hy0[:], stT[:, 1, bs:bs + CH],
                                 wcat[:, 0, 0:512], start=False, stop=True)
                hy1 = psh.tile([CH, T], F32, tag="hy1")
                nc.tensor.matmul(hy1[:], ones2[:], biascat[:, 512:NW],
                                 start=True, stop=False)
                nc.tensor.matmul(hy1[:], stT[:, 0, bs:bs + CH],
                                 wcat[:, 0, 512:NW], start=False, stop=False)
                nc.tensor.matmul(hy1[:], stT[:, 0, bs:bs + CH],
                                 wcat[:, 1, 512:NW], start=False, stop=False)
                nc.tensor.matmul(hy1[:], stT[:, 1, bs:bs + CH],
                                 wcat[:, 0, 512:NW], start=False, stop=True)

                habs = smp.tile([CH, 512], F32, tag="habs")
                nc.scalar.activation(habs[:], hy0[:],
                                     mybir.ActivationFunctionType.Abs)
                nc.scalar.copy(v_all[:, ci, :], hy1[:])

                # Lraw[b,q,k] = x[b,q] * x[b,k] in fp16; every operand's last
                # AP dim is a packed [1,2] pair -> 2x DVE mode.
                L = bigp.tile([CH, T, T], F16, tag="L")
                xq_b = (xall[:, ci, T:3 * T]
                        .rearrange("p (q two) -> p q two", two=2)
                        .unsqueeze(2).broadcast_to([CH, T, T // 2, 2]))
                xk_b = (xall[:, ci, 0:T].rearrange("p (kh kl) -> p kh kl", kl=2)
                        .unsqueeze(1).broadcast_to([CH, T, T // 2, 2]))
                L_v = L[:].rearrange("p q (kh kl) -> p q kh kl", kl=2)
                nc.vector.tensor_tensor(L_v, xq_b, xk_b,
                                        op=mybir.AluOpType.mult)
                L_tiles.append(L)

                # cr[:,ci,0] = sum wk'*wq' = c (1/sqrt(e) is host-folded),
                # cr[:,ci,1] = sum wv*wo = r, over |hy| so products are >= 0
                # (wcat order [wk|wv|wq|wo|V])
                pq = smp.tile([CH, 256], F32, tag="pq")
                nc.vector.tensor_tensor(
                    pq[:].rearrange("p (g e) -> p g e", g=2),
                    habs[:, 0:256].rearrange("p (g e) -> p g e", g=2),
                    habs[:, 256:512].rearrange("p (g e) -> p g e", g=2),
                    op=mybir.AluOpType.mult)
                nc.vector.tensor_reduce(
                    cr_all[:, ci, :], pq[:].rearrange("p (g e) -> p g e", g=2),
                    axis=mybir.AxisListType.X, op=mybir.AluOpType.add)

                # E = exp(c*Lraw - 40) in bf16, slice 0 of W = [E | EX]
                W = bigp.tile([CH, 2, T, T], BF16, tag="W")
                nc.scalar.activation(W[:, 0], L[:],
                                     mybir.ActivationFunctionType.Exp,
                                     bias=shift[:], scale=cr_all[:, ci, 0:1])
                W_tiles.append(W)

            for ci in range(NCH):
                W = W_tiles[ci]
                # EX[b,q,k] = E * x[b,k] (bf16 x fp16 at 2x), written as
                # the second slice of W = [E | EX] so every fold level is a
                # single instruction over both paths
                xk2_b = (xall[:, ci, 0:T]
                         .rearrange("p (kh kl) -> p kh kl", kl=2)
                         .unsqueeze(1).broadcast_to([CH, T, T // 2, 2]))
                EX_v = W[:, 1].rearrange("p q (kh kl) -> p q kh kl", kl=2)
                E_v = W[:, 0].rearrange("p q (kh kl) -> p q kh kl", kl=2)
                nc.vector.tensor_tensor(EX_v, E_v, xk2_b,
                                        op=mybir.AluOpType.mult)

                # segmented row sums: joint bf16 half-folds of [E | EX],
                # then one joint fp32 reduce of the two 8-wide tails
                f1 = bigp.tile([CH, 2, T, 32], BF16, tag="f1")
                nc.vector.tensor_tensor(f1[:], W[:, :, :, 0:32],
                                        W[:, :, :, 32:64],
                                        op=mybir.AluOpType.add)
                f2 = smp.tile([CH, 2, T, 16], BF16, tag="f2")
                nc.vector.tensor_tensor(f2[:], f1[:, :, :, 0:16],
                                        f1[:, :, :, 16:32],
                                        op=mybir.AluOpType.add)
                f3 = smp.tile([CH, 2, T, 8], BF16, tag="f3")
                nc.vector.tensor_tensor(f3[:], f2[:, :, :, 0:8],
                                        f2[:, :, :, 8:16],
                                        op=mybir.AluOpType.add)
                nc.vector.tensor_reduce(dnm_all[:, :, ci, :], f3[:],
                                        axis=mybir.AxisListType.X,
                                        op=mybir.AluOpType.add)

                if ci % 2 == 1:
                    tail_group(ci // 2)

    nc.compile()
    return nc


def _split_bf16(a):
    """hi+res bf16 decomposition: a ~ hi + res with both parts bf16."""
    import ml_dtypes
    hi = a.astype(ml_dtypes.bfloat16)
    res = (a - hi.astype(np.float32)).astype(ml_dtypes.bfloat16)
    return hi, res


def kernel(**inputs):
    global _last_in_maps
    nc = _compiled.get("nc")
    if nc is None:
        nc = _compiled["nc"] = _build()

    x = np.ascontiguousarray(np.asarray(inputs["x"], dtype=np.float32)
                             .reshape(B_FULL, T))
    state = np.asarray(inputs["state"], dtype=np.float32)
    x16 = x.astype(np.float16)
    # xall rows: [x | x duplicated pairwise]
    xall = np.concatenate([x16, np.repeat(x16, 2, axis=1)], axis=1)
    stT = np.ascontiguousarray(state.T)    # (D, B_FULL)

    # column order [wk | wv | wq | wo | V]; fold the softmax 1/sqrt(e)
    # into wk and wq (and their biases) so c needs no extra scaling.
    s4 = float(128.0 ** 0.25)
    wcatT = np.concatenate(
        [np.asarray(inputs["wk_w"], np.float32).T / s4,
         np.asarray(inputs["wv_w"], np.float32).T,
         np.asarray(inputs["wq_w"], np.float32).T / s4,
         np.asarray(inputs["wo_w"], np.float32).T,
         np.asarray(inputs["V_w"], np.float32).T], axis=1)
    biascat = np.concatenate(
        [np.asarray(inputs["wk_b"], np.float32) / s4,
         np.asarray(inputs["wv_b"], np.float32),
         np.asarray(inputs["wq_b"], np.float32) / s4,
         np.asarray(inputs["wo_b"], np.float32),
         np.asarray(inputs["V_b"], np.float32)])[None, :]

    w_hi, w_res = _split_bf16(wcatT)
    b_hi, b_res = _split_bf16(biascat)
    wcat2 = np.ascontiguousarray(np.stack([w_hi, w_res]))       # (2, D, NW)
    bias2 = np.ascontiguousarray(
        np.concatenate([b_hi, b_res], axis=0))                  # (2, NW)
    import ml_dtypes
    ones2 = np.ones((2, CH), ml_dtypes.bfloat16)

    in_maps = []
    for i in range(N_CORES):
        sl = slice(i * BC, (i + 1) * BC)
        sT_hi, sT_res = _split_bf16(stT[:, sl])
        in_maps.append({
            "xall": np.ascontiguousarray(xall[sl]),
            "stT": np.ascontiguousarray(np.stack([sT_hi, sT_res])),
            "wcatT": wcat2,
            "biascat": bias2,
            "ones2": ones2,
        })
    _last_in_maps = in_maps

    res = run_bass_kernel_spmd(nc, in_maps, core_ids=list(range(N_CORES)))
    out = np.concatenate([res.results[i]["out"] for i in range(N_CORES)],
                         axis=0)
    return out.reshape(B_FULL, 1, T)
